# revision 21
# baseline (speedup 1.0000x reference)
"""Trainium2 Bass kernel for AlignedLinear (irreps 0e+1o+2e, mul 128).

y[n, o*9+m] = alpha * sum_i x[n, i*9+m] * K[irrep(m), i, o]

Strategy (data-parallel over nodes, 8 cores, all math in fp32 — matmul
rel err vs the fp32 reference is ~6e-7):
  - shard the node axis exactly: 50000 = 8 * 6250; each core runs 12
    tiles of 512 nodes plus one ragged 106-row tile (no padding).
  - per tile: one batched DMA loads x naturally ([128n x 4*1152], the 4
    row-blocks side by side in the free dim); the PE transposes the 9
    stride-9 m-slices into xT_m [128i x 512n] (exact, via identity);
    DVE/ACT copy them out of PSUM.
  - per 128-row block: 9 fp32 matmuls with xT as the stationary operand
    and the (alpha-prescaled) kernel as the moving operand write
    y_block = xT^T @ K directly in n-major layout into one 3-bank PSUM
    tensor; a single strided interleaving copy, split into parallel
    DVE/ACT halves, assembles the [n, (o,m)] output tile; contiguous
    DMAs store it.
  - load DMAs issue on the SP HWDGE queue, store DMAs on the ACT HWDGE
    queue, so a store waiting for its tile never head-of-line-blocks the
    next tile's load issue.
Measured on 8 axon-tunneled trn2 cores: ~206 us HW exec (single-core
NTFF profile), vs a ~190 us DMA floor (58 MB of HBM traffic per core at
~300-320 GB/s effective with both cores of each HBM pair active).
"""

import os

import numpy as np

N_NODES = 50000
DIM = 1152
MUL = 128
NMDIM = 9  # total irrep dim (1+3+5)
IRREP_OF_M = [0, 1, 1, 1, 2, 2, 2, 2, 2]
ALPHA = float(np.sqrt(1.0 / MUL))
N_CORES = 8
TILE_N = 512
PER_CORE = N_NODES // N_CORES  # 6250, exact
FULL_TILES = PER_CORE // TILE_N  # 12
TAIL_ROWS = PER_CORE - FULL_TILES * TILE_N  # 106

_cache = {}

LAST_RESULTS = None  # BassKernelResults of the most recent run (for test.py)


def _install_trace_support():
    """Make trace=True work under axon: inject the missing
    antenv.axon_hooks module and neuter the S3 artifact upload."""
    import contextlib
    import ctypes
    import sys
    import types

    if "antenv.axon_hooks" not in sys.modules:
        mod = types.ModuleType("antenv.axon_hooks")

        def _make_hook():
            try:
                lib = ctypes.CDLL("/opt/axon/libaxon_pjrt.so")
            except OSError:
                return None
            if not hasattr(lib, "axon_start_nrt_profile"):
                return None
            lib.axon_start_nrt_profile.argtypes = [
                ctypes.POINTER(ctypes.c_int64),
                ctypes.c_size_t,
            ]
            lib.axon_start_nrt_profile.restype = ctypes.c_int64
            lib.axon_stop_nrt_profile.argtypes = [ctypes.c_char_p]
            lib.axon_stop_nrt_profile.restype = ctypes.c_int64

            @contextlib.contextmanager
            def _hook(output_dir, device_ids):
                import jax

                jax.devices()
                if device_ids:
                    ids = (ctypes.c_int64 * len(device_ids))(*device_ids)
                    rc = lib.axon_start_nrt_profile(ids, len(device_ids))
                else:
                    rc = lib.axon_start_nrt_profile(None, 0)
                if rc != 0:
                    raise RuntimeError(f"axon_start_nrt_profile rc={rc}")
                try:
                    yield
                finally:
                    n = lib.axon_stop_nrt_profile(str(output_dir).encode())
                    print(f"ntff profile: {n} file(s) -> {output_dir}")

            return _hook

        hook = _make_hook()
        mod.get_axon_ntff_profile_hook = lambda: hook
        mod.set_axon_ntff_profile_hook = lambda h: None
        sys.modules["antenv.axon_hooks"] = mod

    import concourse.bass_utils as bass_utils

    bass_utils.upload_artifacts = lambda tmpdir: tmpdir


def _build():
    import concourse.mybir as mybir
    import concourse.tile as tile
    from concourse import bacc
    from concourse.masks import make_identity

    F32 = mybir.dt.float32

    nc = bacc.Bacc("TRN2", target_bir_lowering=False)
    x = nc.dram_tensor("x", [PER_CORE, DIM], F32, kind="ExternalInput").ap()
    kern = nc.dram_tensor("kern", [3, MUL, MUL], F32, kind="ExternalInput").ap()
    y = nc.dram_tensor("y", [PER_CORE, DIM], F32, kind="ExternalOutput").ap()

    with tile.TileContext(nc) as tc:
        with (
            tc.tile_pool(name="const", bufs=1) as const_pool,
            tc.tile_pool(name="xin", bufs=3) as xin_pool,
            tc.tile_pool(name="xts", bufs=12) as xts_pool,
            tc.tile_pool(name="yout", bufs=4) as yout_pool,
            tc.tile_pool(name="xtp", bufs=2, space="PSUM") as xtp_pool,
            tc.tile_pool(name="yps", bufs=2, space="PSUM") as yps_pool,
        ):
            def jblocks(nrows):
                return [
                    (j * MUL, min(MUL, nrows - j * MUL))
                    for j in range((nrows + MUL - 1) // MUL)
                ]

            def emit_loads(nbase, nrows, split=False):
                # one batched DMA brings the whole 512-row tile (4 j-blocks
                # side by side in the free dim); split=True issues per-block
                # DMAs instead so the first transpose can start sooner
                nj = len(jblocks(nrows))
                xbig = xin_pool.tile([MUL, 4 * DIM], F32, tag="xin", name="xin")
                if nrows != TILE_N:
                    assert nj == 1
                    nc.sync.dma_start(
                        out=xbig[:nrows, :DIM],
                        in_=x[nbase : nbase + nrows, :],
                    )
                elif split:
                    for off, rows in jblocks(nrows):
                        jj = off // MUL
                        nc.sync.dma_start(
                            out=xbig[:, jj * DIM : (jj + 1) * DIM],
                            in_=x[nbase + off : nbase + off + rows, :],
                        )
                else:
                    nc.sync.dma_start(
                        out=xbig[:].rearrange("p (j d) -> p j d", j=4),
                        in_=x[nbase : nbase + TILE_N, :].rearrange(
                            "(j p) d -> p j d", p=MUL
                        ),
                    )
                return [
                    (off, rows, xbig[:, (off // MUL) * DIM : (off // MUL + 1) * DIM])
                    for off, rows in jblocks(nrows)
                ]

            tiles = [(t * TILE_N, TILE_N) for t in range(FULL_TILES)]
            if TAIL_ROWS:
                tiles.append((FULL_TILES * TILE_N, TAIL_ROWS))

            # first tile's loads go first so SP starts streaming x
            # before the constants
            blocks0 = emit_loads(*tiles[0], split=True)

            ident = const_pool.tile([MUL, MUL], F32)
            make_identity(nc, ident[:])
            # kern_sb: [i=128, (t,o)=384], pre-scaled by alpha
            kern_sb = const_pool.tile([MUL, 3 * MUL], F32)
            nc.sync.dma_start(
                out=kern_sb[:].rearrange("i (t o) -> i t o", t=3),
                in_=kern.rearrange("t i o -> i t o"),
            )
            nc.vector.tensor_scalar_mul(kern_sb[:], kern_sb[:], ALPHA)

            for ti, (nbase, nrows) in enumerate(tiles):
                blocks = blocks0 if ti == 0 else emit_loads(nbase, nrows)
                tot = nrows

                # transpose all 9 m-slices: xT_m [i=128, n=tot]
                xt_all = []
                for m in range(NMDIM):
                    xtp = xtp_pool.tile(
                        [MUL, TILE_N], F32, tag="xtp", name="xtp"
                    )
                    for off, rows, xsb in blocks:
                        nc.tensor.transpose(
                            xtp[:, off : off + rows],
                            xsb[:rows, :].rearrange(
                                "p (i m) -> p m i", m=NMDIM
                            )[:, m, :],
                            ident[:rows, :rows],
                        )
                    xt_sb = xts_pool.tile(
                        [MUL, TILE_N], F32, tag="xts", name="xts"
                    )
                    if m % 2 == 0:
                        nc.vector.tensor_copy(xt_sb[:, :tot], xtp[:, :tot])
                    else:
                        nc.scalar.copy(xt_sb[:, :tot], xtp[:, :tot])
                    xt_all.append(xt_sb)

                # per n-block: 9 matmuls into one 3-bank PSUM tensor, then
                # the split assembly copy and the store DMA
                for off, rows, _ in blocks:
                    jtag = off // MUL
                    yp = yps_pool.tile(
                        [MUL, NMDIM * MUL], F32, tag="yps", name="yps"
                    )
                    for m in range(NMDIM):
                        ks = IRREP_OF_M[m] * MUL
                        nc.tensor.matmul(
                            yp[:rows, m * MUL : (m + 1) * MUL],
                            xt_all[m][:, off : off + rows],
                            kern_sb[:, ks : ks + MUL],
                            start=True,
                            stop=True,
                        )
                    out_sb = yout_pool.tile(
                        [MUL, DIM], F32, tag=f"yout{jtag}", name=f"yout{jtag}"
                    )
                    # split the interleaving copy across DVE and ACT so the
                    # two halves run in parallel (shorter convoy, balanced)
                    dst = out_sb[:rows].rearrange("p (o m) -> p o m", o=MUL)
                    src = yp[:rows].rearrange("p (m o) -> p o m", o=MUL)
                    half = MUL // 2
                    nc.vector.tensor_copy(dst[:, :half, :], src[:, :half, :])
                    nc.scalar.copy(dst[:, half:, :], src[:, half:, :])
                    # stores go out the ACT HWDGE queue so a store waiting on
                    # out_sb never blocks the next tile's x-load issue on SP
                    nc.scalar.dma_start(
                        out=y[nbase + off : nbase + off + rows, :],
                        in_=out_sb[:rows],
                    )

    nc.compile()
    return nc


def _run_on_device(x, kern):
    from concourse import bass_utils

    if "nc" not in _cache:
        _cache["nc"] = _build()
    nc = _cache["nc"]

    in_maps = [
        {"x": x[c * PER_CORE : (c + 1) * PER_CORE], "kern": kern}
        for c in range(N_CORES)
    ]

    trace = os.environ.get("KERNEL_TRACE", "0") == "1"
    if trace:
        _install_trace_support()
    res = bass_utils.run_bass_kernel_spmd(
        nc, in_maps, core_ids=list(range(N_CORES)), trace=trace
    )
    global LAST_RESULTS
    LAST_RESULTS = res

    out = np.concatenate([res.results[c]["y"] for c in range(N_CORES)], axis=0)
    return np.ascontiguousarray(out[:N_NODES])


def _run_in_subprocess(x, kern):
    """Recovery path: a rare transient NRT_EXEC_UNIT_UNRECOVERABLE poisons
    the in-process PJRT client; a fresh process gets a fresh client and
    succeeds. Ship inputs/outputs through a temp npz."""
    import subprocess
    import sys
    import tempfile

    d = tempfile.mkdtemp()
    inp = os.path.join(d, "in.npz")
    outp = os.path.join(d, "out.npy")
    np.savez(inp, x=x, kernel=kern)
    code = (
        "import numpy as np, sys; sys.path.insert(0, %r); "
        "import os; os.environ['KERNEL_TRACE']='0'; "
        "import kernel as km; z = np.load(%r); "
        "y = km._run_on_device(z['x'], z['kernel']); np.save(%r, y)"
        % (os.path.dirname(os.path.abspath(__file__)), inp, outp)
    )
    subprocess.run([sys.executable, "-c", code], check=True, timeout=900)
    return np.load(outp)


def _spot_check(x, kern, y, n_rows=16):
    """Cheap host-side sanity check on a few random rows (guards against
    a silently corrupted device run)."""
    rng = np.random.default_rng(12345)
    rows = rng.choice(x.shape[0], size=n_rows, replace=False)
    kernel_rep = np.repeat(kern.astype(np.float64), [1, 3, 5], axis=0)
    xs = x[rows].astype(np.float64).reshape(n_rows, MUL, NMDIM)
    want = ALPHA * np.einsum("nim,mio->nom", xs, kernel_rep)
    want = want.reshape(n_rows, DIM)
    got = y[rows].astype(np.float64)
    scale = max(np.abs(want).max(), 1e-12)
    return np.abs(got - want).max() / scale < 1e-4


def kernel(**inputs):
    x = np.ascontiguousarray(np.asarray(inputs["x"], dtype=np.float32))
    kern = np.ascontiguousarray(np.asarray(inputs["kernel"], dtype=np.float32))

    try:
        y = _run_on_device(x, kern)
        if _spot_check(x, kern, y):
            return y
    except Exception:
        pass
    return _run_in_subprocess(x, kern)
